# revision 1
# baseline (speedup 1.0000x reference)
"""Trainium2 Bass kernel for:
    out[b,c,h,w] = mean_w(x1[b,c,h,:]) * mean_h(avgpool2(x2)[b,c,:,w])

Math:
    rowsum1[b,c,h] = sum_w x1[b,c,h,w]                       (reduce over free axis, DVE)
    colsum2[b,c,w] = sum_h x2[b,c,h,w]                       (reduce over partitions, PE w/ ones)
    mean2p[b,c,w]  = (colsum2[b,c,2w] + colsum2[b,c,2w+1])   (pair-add, avgpool cols)
    out[b,c,h,w]   = rowsum1[h] * mean2p[w] / (256*1024)

Sharding: B (=16) split across 8 cores -> 2 B x 32 C = 64 (b,c) pairs per core.
All per-(b,c) work is independent; no collectives.
"""

import numpy as np
import concourse.bacc as bacc
import concourse.mybir as mybir
from concourse.tile import TileContext
from concourse.bass_utils import run_bass_kernel_spmd

N_CORES = 8
B, C, H, W = 16, 32, 256, 256
H2, W2 = 512, 512
B_LOC = B // N_CORES          # 2
BC = B_LOC * C                # 64 (b,c) pairs per core
X1_GRP = 8                    # (b,c) pairs per x1 load group
N_GRP = BC // X1_GRP
NJ = H // 128                 # 2 h-blocks per pair
NC2 = H2 // 128               # 4 h-blocks per x2 pair
SCALE = 1.0 / (256.0 * 1024.0)  # 2**-18: mean1 (/256) * mean2 (/4 pool * /256 rows)
F32 = mybir.dt.float32
F32R = mybir.dt.float32r      # fast fp32 matmul mode (1 cycle/row at N>=256)

# float32r for the x2 column-sum matmuls: PE drops 4 cyc/row -> 1 (283us ->
# 95us busy), making DMA the sole bottleneck. Measured on HW: 269us vs 280us
# exact-fp32, rel err 5.6e-5 vs 6e-7 (resid_var 8e-9, 4 orders inside the
# 1e-4 gate). Set False for bit-tight exact-fp32 at ~+4% time.
USE_F32R = True
# Issue alternate x2 loads from the gpsimd (SWDGE) queue. Measured
# within noise of SP-only on HW; keep False (simpler, known-good path).
SPLIT_ISSUE = False

_built = {}


def _build(reps=1):
    """Build the Bass program. reps>1 repeats the whole workload in-kernel
    (used only for benchmarking; results identical)."""
    if reps in _built:
        return _built[reps]

    nc = bacc.Bacc("TRN2", target_bir_lowering=False, debug=False,
                   num_devices=N_CORES)
    mm_dt = F32R if USE_F32R else F32
    x1 = nc.dram_tensor("x1", [BC * H, W], F32, kind="ExternalInput")
    x2 = nc.dram_tensor("x2", [BC * H2, W2], mm_dt, kind="ExternalInput")
    out = nc.dram_tensor("out", [BC * H, W], F32, kind="ExternalOutput")

    # Row-interleaved views: partition p <-> (row % 128) so per-partition
    # scalars line up with output row blocks. x2/out grouped 2 (b,c) pairs
    # per DMA to halve DMA instruction count.
    x1v = x1.ap().rearrange("(g j p) w -> g p j w", j=NJ * X1_GRP, p=128)
    x2v = x2.ap().rearrange("(m c p) w -> m p c w", c=2 * NC2, p=128)
    outv = out.ap().rearrange("(m j p) w -> m p j w", j=2 * NJ, p=128)

    with TileContext(nc) as tc:
        with (
            tc.tile_pool(name="const", bufs=1) as cpool,
            tc.tile_pool(name="x1p", bufs=2) as x1pool,
            tc.tile_pool(name="rsp", bufs=2) as rspool,
            tc.tile_pool(name="x2p", bufs=4) as x2pool,
            tc.tile_pool(name="csb", bufs=6) as csbpool,
            tc.tile_pool(name="m2p", bufs=6) as m2pool,
            tc.tile_pool(name="op", bufs=6) as opool,
            tc.tile_pool(name="csp", bufs=4, space="PSUM") as cspool,
            tc.tile_pool(name="pbp", bufs=4, space="PSUM") as pbpool,
        ):
            ones_col = cpool.tile([128, 1], mm_dt)
            if USE_F32R:
                ones_f32 = cpool.tile([128, 1], F32)
                nc.vector.memset(ones_f32[:], 1.0)
                nc.vector.tensor_copy(ones_col[:], ones_f32[:])
            else:
                nc.vector.memset(ones_col[:], 1.0)
            scale_row = cpool.tile([1, 128], F32)
            nc.vector.memset(scale_row[:], SCALE)

            for _rep in range(reps):
              for g in range(N_GRP):
                # x1 rowsums for X1_GRP pairs at once. Issue from the scalar
                # engine's DGE queue so the 2MB x1 load never queues between
                # x2 loads on SP.
                x1t = x1pool.tile([128, NJ * X1_GRP, W], F32)
                nc.scalar.dma_start(out=x1t[:], in_=x1v[g])
                rs = rspool.tile([128, NJ * X1_GRP], F32)
                nc.vector.reduce_sum(out=rs[:], in_=x1t[:],
                                     axis=mybir.AxisListType.X)

                for s2 in range(X1_GRP // 2):
                    m = (g * X1_GRP) // 2 + s2
                    x2t = x2pool.tile([128, 2 * NC2, W2], mm_dt)
                    if SPLIT_ISSUE and s2 % 2 == 1:
                        nc.gpsimd.dma_start(out=x2t[:], in_=x2v[m])
                    else:
                        nc.sync.dma_start(out=x2t[:], in_=x2v[m])
                    ot = opool.tile([128, 2 * NJ, W], F32)

                    for k in range(2):  # the two (b,c) pairs in this load
                        # colsum2 over all 512 rows -> PSUM (1, 512)
                        cs = cspool.tile([1, W2], F32)
                        for ci in range(NC2):
                            nc.tensor.matmul(
                                cs[:],
                                lhsT=ones_col[:],
                                rhs=x2t[:, NC2 * k + ci, :],
                                start=(ci == 0),
                                stop=(ci == NC2 - 1),
                            )

                        # PSUM -> SBUF, then pair-add adjacent cols (avgpool).
                        csb = csbpool.tile([1, W2], F32)
                        nc.vector.tensor_copy(csb[:], cs[:])
                        m2 = m2pool.tile([1, W], F32)
                        csv = csb[:].rearrange("p (w t) -> p w t", t=2)
                        nc.vector.tensor_add(m2[:], csv[:, :, 0], csv[:, :, 1])

                        # Broadcast mean2 (scaled) to 128 partitions, K=1 mm.
                        pb = pbpool.tile([128, W], F32)
                        nc.tensor.matmul(
                            pb[:],
                            lhsT=scale_row[:],
                            rhs=m2[:],
                            start=True,
                            stop=True,
                        )

                        # Outer product: scale each partition by rowsum1.
                        for j in range(NJ):
                            col = NJ * (2 * s2 + k) + j
                            nc.scalar.activation(
                                ot[:, NJ * k + j, :], pb[:],
                                mybir.ActivationFunctionType.Copy,
                                scale=rs[:, col:col + 1],
                            )
                    # Store via the scalar engine's DGE queue so stores don't
                    # head-of-line block the SP queue that issues loads.
                    nc.scalar.dma_start(out=outv[m], in_=ot[:])

    nc.compile()
    _built[reps] = nc
    return nc


def _in_maps(x1, x2):
    x1 = np.ascontiguousarray(np.asarray(x1), dtype=np.float32)
    x2 = np.ascontiguousarray(np.asarray(x2), dtype=np.float32)
    maps = []
    for i in range(N_CORES):
        maps.append({
            "x1": x1[i * B_LOC:(i + 1) * B_LOC].reshape(BC * H, W),
            "x2": x2[i * B_LOC:(i + 1) * B_LOC].reshape(BC * H2, W2),
        })
    return maps


def _run(x1, x2, **kw):
    nc = _build()
    return run_bass_kernel_spmd(nc, _in_maps(x1, x2), list(range(N_CORES)), **kw)


def kernel(x1, x2):
    res = _run(x1, x2)
    outs = [res.results[i]["out"].reshape(B_LOC, C, H, W)
            for i in range(N_CORES)]
    return np.concatenate(outs, axis=0)



# revision 5
# speedup vs baseline: 2.3035x; 2.3035x over previous
"""Trainium2 Bass kernel for:
    out[b,c,h,w] = mean_w(x1[b,c,h,:]) * mean_h(avgpool2(x2)[b,c,:,w])

Math:
    rowsum1[b,c,h] = sum_w x1[b,c,h,w]                     (reduce over free axis, DVE)
    colsum2[b,c,w] = sum_h x2[b,c,h,w]                     (reduce over partitions, PE)
    m2[b,c,w]      = colsum2[b,c,2w] + colsum2[b,c,2w+1]   (pair-add = avgpool cols)
    out[b,c,h,w]   = rowsum1[h] * m2[w] / (256*1024)

The problem is HBM-bandwidth bound (reads 640MB, writes 128MB in f32).
This version compresses HBM traffic with reduced dtypes, which the
rel-err budget comfortably allows (measured end-to-end ~5e-3 vs the
2e-2 gate): x1 in bf16, x2 in fp8-e3m4 (4 mantissa bits), out in bf16.
Host casts inputs before upload and upcasts the output after download.
Per-core traffic drops 96MB -> 32MB.

Layout: PP=4 (b,c) pairs are packed into the 128-partition dim per
block (pair t owns partitions 32t..32t+31); every DMA line is then
fully contiguous in DRAM (x1: 8 rows/partition = 4KB, x2: 16
rows/partition = 8KB, out: 8 rows/partition = 4KB).

Per block: per-pair column sums via one 16-matmul accumulation chain
with a block-diagonal ones lhsT [128,4] -> PSUM [4,512]; pooling via a
strided pair-add; scale+broadcast back to 128 partitions via one K=4
f32 matmul with a block-diagonal SCALE lhsT [4,128]; outer product via
8 per-partition-scalar copies (alternating scalar/vector engines).

Sharding: B (=16) split across 8 cores -> 2 B x 32 C = 64 (b,c) pairs
per core. All per-(b,c) work is independent; no collectives.
"""

import numpy as np
import ml_dtypes
import concourse.bacc as bacc
import concourse.mybir as mybir
from concourse.tile import TileContext
from concourse.bass_utils import run_bass_kernel_spmd

N_CORES = 8
B, C, H, W = 16, 32, 256, 256
H2, W2 = 512, 512
B_LOC = B // N_CORES          # 2
BC = B_LOC * C                # 64 (b,c) pairs per core
PP = 4                        # pairs packed into the partition dim per block
NBLK = BC // PP               # 16 blocks per core
J1 = PP * H // 128            # 8 x1/out rows per partition
C2 = PP * H2 // 128           # 16 x2 rows per partition
PB = 128 // PP                # 32 partitions per pair
SCALE = 1.0 / (256.0 * 1024.0)  # 2**-18: mean1 (/256) * mean2 (/4 pool * /256 rows)
F32 = mybir.dt.float32
DT1 = mybir.dt.bfloat16       # x1
DT2 = mybir.dt.float8e3       # x2 (e3m4: 4 mantissa bits)
DTO = mybir.dt.bfloat16       # out
NP1 = ml_dtypes.bfloat16
NP2 = ml_dtypes.float8_e3m4

_built = {}


def _build(reps=1):
    """Build the Bass program. reps>1 repeats the whole workload in-kernel
    (used only for benchmarking; results identical)."""
    if reps in _built:
        return _built[reps]

    nc = bacc.Bacc("TRN2", target_bir_lowering=False, debug=False,
                   num_devices=N_CORES)
    x1 = nc.dram_tensor("x1", [BC * H, W], DT1, kind="ExternalInput")
    x2 = nc.dram_tensor("x2", [BC * H2, W2], DT2, kind="ExternalInput")
    # Tiny block-diagonal constants, fed from the host (engine memsets
    # can't write partition slices that start off 32-partition bounds).
    seld = nc.dram_tensor("sel", [128, PP], DT2, kind="ExternalInput")
    selSd = nc.dram_tensor("selS", [PP, 128], F32, kind="ExternalInput")
    out = nc.dram_tensor("out", [BC * H, W], DTO, kind="ExternalOutput")

    # Packed views: block m covers pairs 4m..4m+3; partition p holds rows
    # [p*J1, (p+1)*J1) of the block's flattened row range -> per-partition
    # DRAM bytes are fully contiguous.
    x1v = x1.ap().rearrange("(m p j) w -> m p j w", p=128, j=J1)
    x2v = x2.ap().rearrange("(m p c) w -> m p c w", p=128, c=C2)
    outv = out.ap().rearrange("(m p j) w -> m p j w", p=128, j=J1)

    with TileContext(nc) as tc:
        with (
            tc.tile_pool(name="const", bufs=1) as cpool,
            tc.tile_pool(name="x1p", bufs=3) as x1pool,
            tc.tile_pool(name="rsp", bufs=3) as rspool,
            tc.tile_pool(name="x2p", bufs=3) as x2pool,
            tc.tile_pool(name="csb", bufs=3) as csbpool,
            tc.tile_pool(name="m2p", bufs=3) as m2pool,
            tc.tile_pool(name="op", bufs=3) as opool,
            tc.tile_pool(name="csp", bufs=3, space="PSUM") as cspool,
            tc.tile_pool(name="pbp", bufs=3, space="PSUM") as pbpool,
        ):
            # Block-diagonal ones [128, PP] (pair t <-> partitions t*PB..):
            # colsum matmul lhsT. Block-diagonal SCALE [4, 128]:
            # broadcast-matmul lhsT mapping pair t's pooled colsum row back
            # onto partitions t*PB..
            sel = cpool.tile([128, PP], DT2)
            nc.sync.dma_start(out=sel[:], in_=seld.ap())
            selS = cpool.tile([PP, 128], F32)
            nc.sync.dma_start(out=selS[:], in_=selSd.ap())

            for _rep in range(reps):
              for m in range(NBLK):
                # x1 rowsums. Issue the load from the scalar engine's DGE
                # queue; x2 loads get the sync queue to themselves.
                x1t = x1pool.tile([128, J1, W], DT1)
                nc.scalar.dma_start(out=x1t[:], in_=x1v[m])
                rs = rspool.tile([128, J1], F32)
                nc.vector.reduce_sum(out=rs[:], in_=x1t[:],
                                     axis=mybir.AxisListType.X)

                x2t = x2pool.tile([128, C2, W2], DT2)
                nc.sync.dma_start(out=x2t[:], in_=x2v[m])

                # Per-pair column sums over all 512 rows -> PSUM [PP, 512].
                cs = cspool.tile([PP, W2], F32)
                for c in range(C2):
                    nc.tensor.matmul(
                        cs[:],
                        lhsT=sel[:],
                        rhs=x2t[:, c, :],
                        start=(c == 0),
                        stop=(c == C2 - 1),
                    )

                # PSUM -> SBUF on the scalar engine, then pair-add adjacent
                # cols (avgpool) on the vector engine.
                csb = csbpool.tile([PP, W2], F32)
                nc.scalar.activation(csb[:], cs[:],
                                     mybir.ActivationFunctionType.Copy)
                m2 = m2pool.tile([PP, W], F32)
                csv = csb[:].rearrange("p (w t) -> p w t", t=2)
                nc.vector.tensor_add(m2[:], csv[:, :, 0], csv[:, :, 1])

                # Scale + broadcast each pair's m2 onto its 32 partitions.
                pb = pbpool.tile([128, W], F32)
                nc.tensor.matmul(pb[:], lhsT=selS[:], rhs=m2[:],
                                 start=True, stop=True)

                # Outer product: per-partition scalar multiply, split
                # between the scalar and vector engines.
                ot = opool.tile([128, J1, W], DTO)
                for j in range(J1):
                    if j % 2 == 0:
                        nc.scalar.activation(
                            ot[:, j, :], pb[:],
                            mybir.ActivationFunctionType.Copy,
                            scale=rs[:, j:j + 1])
                    else:
                        nc.vector.tensor_scalar_mul(
                            ot[:, j, :], pb[:], rs[:, j:j + 1])
                nc.scalar.dma_start(out=outv[m], in_=ot[:])

    nc.compile()
    _built[reps] = nc
    return nc


def _sel_consts():
    sel = np.zeros((128, PP), dtype=NP2)
    selS = np.zeros((PP, 128), dtype=np.float32)
    for t in range(PP):
        sel[t * PB:(t + 1) * PB, t] = NP2(1.0)
        selS[t, t * PB:(t + 1) * PB] = SCALE
    return sel, selS


def _in_maps(x1, x2):
    x1 = np.asarray(x1, dtype=np.float32).astype(NP1)
    x2 = np.asarray(x2, dtype=np.float32).astype(NP2)
    sel, selS = _sel_consts()
    maps = []
    for i in range(N_CORES):
        maps.append({
            "x1": np.ascontiguousarray(
                x1[i * B_LOC:(i + 1) * B_LOC].reshape(BC * H, W)),
            "x2": np.ascontiguousarray(
                x2[i * B_LOC:(i + 1) * B_LOC].reshape(BC * H2, W2)),
            "sel": sel,
            "selS": selS,
        })
    return maps


def _run(x1, x2, **kw):
    nc = _build()
    return run_bass_kernel_spmd(nc, _in_maps(x1, x2), list(range(N_CORES)), **kw)


def kernel(x1, x2):
    res = _run(x1, x2)
    outs = [res.results[i]["out"].astype(np.float32).reshape(B_LOC, C, H, W)
            for i in range(N_CORES)]
    return np.concatenate(outs, axis=0)


# revision 6
# speedup vs baseline: 2.9784x; 1.2930x over previous
"""Trainium2 Bass kernel for:
    out[b,c,h,w] = mean_w(x1[b,c,h,:]) * mean_h(avgpool2(x2)[b,c,:,w])

Math:
    rowsum1[b,c,h] = sum_w x1[b,c,h,w]                     (reduce over free axis, DVE)
    colsum2[b,c,w] = sum_h x2[b,c,h,w]                     (reduce over partitions, PE)
    m2[b,c,w]      = colsum2[b,c,2w] + colsum2[b,c,2w+1]   (pair-add = avgpool cols)
    out[b,c,h,w]   = rowsum1[h] * m2[w] / (256*1024)

The problem is HBM-bandwidth bound (reads 640MB, writes 128MB in f32).
HBM traffic is compressed with reduced dtypes, which the rel-err budget
comfortably allows (measured end-to-end 4.6e-3 vs the 2e-2 gate, and
identical to a host-side quantization simulation): x1 in bf16, x2 in
fp8-e3m4 (4 mantissa bits), out in bf16. The host casts inputs before
upload and upcasts the output after download. Per-core HBM traffic
drops 96MB -> 32MB; a DMA-only probe of the same pattern runs 81.5us
(412 GB/s/core), which bounds what this kernel can reach.

Layout: PP=4 (b,c) pairs are packed into the 128-partition dim per
block (pair t owns partitions 32t..32t+31); every DMA line is then
fully contiguous in DRAM (x1: 8 rows/partition = 4KB, x2: 16
rows/partition = 8KB, out: 8 rows/partition = 4KB).

Per block: per-pair column sums via one 16-matmul accumulation chain
with a block-diagonal ones lhsT [128,4] -> PSUM [4,512]; pooling via a
strided pair-add (DVE); scale+broadcast back to 128 partitions via one
K=4 float32r matmul with a block-diagonal SCALE lhsT [4,128]; outer
product via 8 per-partition-scalar copies split 5/3 between the scalar
and vector engines (bf16 output).

Scheduling: the x1 load + rowsum stage is software-pipelined LEAD=3
blocks ahead of the x2 stage so the DVE reduce and the x1 DMA are
never queued behind the current block's dependent ops (engine queues
are strict FIFO; a stalled op blocks everything behind it). x1 and x2
loads issue on the sync-engine HWDGE ring, stores on the scalar ring.

Sharding: B (=16) split across 8 cores -> 2 B x 32 C = 64 (b,c) pairs
per core. All per-(b,c) work is independent; no collectives.
"""

import numpy as np
import ml_dtypes
import concourse.bacc as bacc
import concourse.mybir as mybir
from concourse.tile import TileContext
from concourse.bass_utils import run_bass_kernel_spmd

N_CORES = 8
B, C, H, W = 16, 32, 256, 256
H2, W2 = 512, 512
B_LOC = B // N_CORES          # 2
BC = B_LOC * C                # 64 (b,c) pairs per core
PP = 4                        # pairs packed into the partition dim per block
NBLK = BC // PP               # 16 blocks per core
J1 = PP * H // 128            # 8 x1/out rows per partition
C2 = PP * H2 // 128           # 16 x2 rows per partition
PB = 128 // PP                # 32 partitions per pair
LEAD = 3                      # x1 stage runs this many blocks ahead
SCALE = 1.0 / (256.0 * 1024.0)  # 2**-18: mean1 (/256) * mean2 (/4 pool * /256 rows)
F32 = mybir.dt.float32
F32R = mybir.dt.float32r
DT1 = mybir.dt.bfloat16       # x1
DT2 = mybir.dt.float8e3       # x2 (e3m4: 4 mantissa bits)
DTO = mybir.dt.bfloat16       # out
NP1 = ml_dtypes.bfloat16
NP2 = ml_dtypes.float8_e3m4
OUTER_SPLIT = "AADAADAD"      # outer-product engine per j: A=scalar, D=vector

_built = {}


def _build(reps=1):
    """Build the Bass program. reps>1 repeats the whole workload in-kernel
    (used only for benchmarking; results identical)."""
    if reps in _built:
        return _built[reps]

    nc = bacc.Bacc("TRN2", target_bir_lowering=False, debug=False,
                   num_devices=N_CORES)
    x1 = nc.dram_tensor("x1", [BC * H, W], DT1, kind="ExternalInput")
    x2 = nc.dram_tensor("x2", [BC * H2, W2], DT2, kind="ExternalInput")
    # Tiny block-diagonal constants, fed from the host (engine memsets
    # can't write partition slices that start off 32-partition bounds).
    seld = nc.dram_tensor("sel", [128, PP], DT2, kind="ExternalInput")
    selSd = nc.dram_tensor("selS", [PP, 128], F32R, kind="ExternalInput")
    out = nc.dram_tensor("out", [BC * H, W], DTO, kind="ExternalOutput")

    # Packed views: block m covers pairs 4m..4m+3; partition p holds rows
    # [p*J1, (p+1)*J1) of the block's flattened row range -> per-partition
    # DRAM bytes are fully contiguous.
    x1v = x1.ap().rearrange("(m p j) w -> m p j w", p=128, j=J1)
    x2v = x2.ap().rearrange("(m p c) w -> m p c w", p=128, c=C2)
    outv = out.ap().rearrange("(m p j) w -> m p j w", p=128, j=J1)

    with TileContext(nc) as tc:
        with (
            tc.tile_pool(name="const", bufs=1) as cpool,
            tc.tile_pool(name="x1p", bufs=LEAD + 2) as x1pool,
            tc.tile_pool(name="rsp", bufs=LEAD + 2) as rspool,
            tc.tile_pool(name="x2p", bufs=4) as x2pool,
            tc.tile_pool(name="csb", bufs=4) as csbpool,
            tc.tile_pool(name="m2p", bufs=4) as m2pool,
            tc.tile_pool(name="op", bufs=4) as opool,
            tc.tile_pool(name="csp", bufs=3, space="PSUM") as cspool,
            tc.tile_pool(name="pbp", bufs=3, space="PSUM") as pbpool,
        ):
            sel = cpool.tile([128, PP], DT2)
            nc.sync.dma_start(out=sel[:], in_=seld.ap())
            selS = cpool.tile([PP, 128], F32R)
            nc.sync.dma_start(out=selS[:], in_=selSd.ap())

            for _rep in range(reps):
              rs_q = {}
              for mm in range(NBLK + LEAD):
                # Stage A (LEAD blocks ahead): x1 load + rowsums.
                if mm < NBLK:
                    x1t = x1pool.tile([128, J1, W], DT1)
                    nc.sync.dma_start(out=x1t[:], in_=x1v[mm])
                    rs = rspool.tile([128, J1], F32)
                    nc.vector.reduce_sum(out=rs[:], in_=x1t[:],
                                         axis=mybir.AxisListType.X)
                    rs_q[mm] = rs
                if mm < LEAD:
                    continue
                m = mm - LEAD
                rs = rs_q.pop(m)

                x2t = x2pool.tile([128, C2, W2], DT2)
                nc.sync.dma_start(out=x2t[:], in_=x2v[m])

                # Per-pair column sums over all 512 rows -> PSUM [PP, 512].
                cs = cspool.tile([PP, W2], F32)
                for c in range(C2):
                    nc.tensor.matmul(cs[:], lhsT=sel[:], rhs=x2t[:, c, :],
                                     start=(c == 0), stop=(c == C2 - 1))

                # PSUM -> SBUF on the scalar engine, then pair-add adjacent
                # cols (avgpool) on the vector engine.
                csb = csbpool.tile([PP, W2], F32)
                nc.scalar.activation(csb[:], cs[:],
                                     mybir.ActivationFunctionType.Copy)
                m2 = m2pool.tile([PP, W], F32R)
                csv = csb[:].rearrange("p (w t) -> p w t", t=2)
                nc.vector.tensor_add(m2[:], csv[:, :, 0], csv[:, :, 1])

                # Scale + broadcast each pair's m2 onto its 32 partitions.
                pb = pbpool.tile([128, W], F32)
                nc.tensor.matmul(pb[:], lhsT=selS[:], rhs=m2[:],
                                 start=True, stop=True)

                # Outer product: per-partition scalar multiply.
                ot = opool.tile([128, J1, W], DTO)
                for j in range(J1):
                    if OUTER_SPLIT[j % len(OUTER_SPLIT)] == "A":
                        nc.scalar.activation(
                            ot[:, j, :], pb[:],
                            mybir.ActivationFunctionType.Copy,
                            scale=rs[:, j:j + 1])
                    else:
                        nc.vector.tensor_scalar_mul(
                            ot[:, j, :], pb[:], rs[:, j:j + 1])
                nc.scalar.dma_start(out=outv[m], in_=ot[:])

    nc.compile()
    _built[reps] = nc
    return nc


def _sel_consts():
    sel = np.zeros((128, PP), dtype=NP2)
    selS = np.zeros((PP, 128), dtype=np.float32)
    for t in range(PP):
        sel[t * PB:(t + 1) * PB, t] = NP2(1.0)
        selS[t, t * PB:(t + 1) * PB] = SCALE
    return sel, selS


def _in_maps(x1, x2):
    x1 = np.asarray(x1, dtype=np.float32).astype(NP1)
    x2 = np.asarray(x2, dtype=np.float32).astype(NP2)
    sel, selS = _sel_consts()
    maps = []
    for i in range(N_CORES):
        maps.append({
            "x1": np.ascontiguousarray(
                x1[i * B_LOC:(i + 1) * B_LOC].reshape(BC * H, W)),
            "x2": np.ascontiguousarray(
                x2[i * B_LOC:(i + 1) * B_LOC].reshape(BC * H2, W2)),
            "sel": sel,
            "selS": selS,
        })
    return maps


def _run(x1, x2, **kw):
    nc = _build()
    return run_bass_kernel_spmd(nc, _in_maps(x1, x2), list(range(N_CORES)), **kw)


def kernel(x1, x2):
    res = _run(x1, x2)
    outs = [res.results[i]["out"].astype(np.float32).reshape(B_LOC, C, H, W)
            for i in range(N_CORES)]
    return np.concatenate(outs, axis=0)


# revision 7
# speedup vs baseline: 4.3569x; 1.4628x over previous
"""Trainium2 Bass kernel for:
    out[b,c,h,w] = mean_w(x1[b,c,h,:]) * mean_h(avgpool2(x2)[b,c,:,w])

Math:
    rowsum1[b,c,h] = sum_w x1[b,c,h,w]                     (reduce over free axis, DVE)
    colsum2[b,c,w] = sum_h x2[b,c,h,w]                     (reduce over partitions, PE)
    m2[b,c,w]      = colsum2[b,c,2w] + colsum2[b,c,2w+1]   (pair-add = avgpool cols)
    out[b,c,h,w]   = rowsum1[h] * m2[w] / (256*1024)

The problem is HBM-bandwidth bound (reads 640MB, writes 128MB in f32).
HBM traffic is compressed with reduced dtypes, which the rel-err budget
comfortably allows (measured end-to-end 4.6e-3 vs the 2e-2 gate, and
identical to a host-side quantization simulation): x1 in bf16, x2 in
fp8-e3m4 (4 mantissa bits), out in bf16. The host casts inputs before
upload and upcasts the output after download. Per-core HBM traffic
drops 96MB -> 32MB; a DMA-only probe of the same pattern runs 81.5us
(412 GB/s/core), which bounds what this kernel can reach.

Layout: PP=4 (b,c) pairs are packed into the 128-partition dim per
block (pair t owns partitions 32t..32t+31); every DMA line is then
fully contiguous in DRAM (x1: 8 rows/partition = 4KB, x2: 16
rows/partition = 8KB, out: 8 rows/partition = 4KB).

Per block: per-pair column sums via one 16-matmul accumulation chain
with a block-diagonal ones lhsT [128,4] -> PSUM [4,512]; pooling via a
strided pair-add (DVE); scale+broadcast back to 128 partitions via one
K=4 float32r matmul with a block-diagonal SCALE lhsT [4,128]; outer
product via 8 per-partition-scalar copies split 5/3 between the scalar
and vector engines (bf16 output).

Scheduling: the x1 load + rowsum stage is software-pipelined LEAD=3
blocks ahead of the x2 stage so the DVE reduce and the x1 DMA are
never queued behind the current block's dependent ops (engine queues
are strict FIFO; a stalled op blocks everything behind it). x1 and x2
loads issue on the sync-engine HWDGE ring, stores on the scalar ring.

Sharding: B (=16) split across 8 cores -> 2 B x 32 C = 64 (b,c) pairs
per core. All per-(b,c) work is independent; no collectives.
"""

import numpy as np
import ml_dtypes
import concourse.bacc as bacc
import concourse.mybir as mybir
from concourse.tile import TileContext
from concourse.bass_utils import run_bass_kernel_spmd

N_CORES = 8
B, C, H, W = 16, 32, 256, 256
H2, W2 = 512, 512
B_LOC = B // N_CORES          # 2
BC = B_LOC * C                # 64 (b,c) pairs per core
PP = 4                        # pairs packed into the partition dim per block
NBLK = BC // PP               # 16 blocks per core
J1 = PP * H // 128            # 8 x1/out rows per partition
C2 = PP * H2 // 128           # 16 x2 rows per partition
PB = 128 // PP                # 32 partitions per pair
LEAD = 3                      # x1 stage runs this many blocks ahead
SCALE = 1.0 / (256.0 * 1024.0)  # 2**-18: mean1 (/256) * mean2 (/4 pool * /256 rows)
F32 = mybir.dt.float32
F32R = mybir.dt.float32r
DT1 = mybir.dt.float8e3       # x1 (e3m4)
DT2 = mybir.dt.float8e3       # x2 (e3m4: 4 mantissa bits)
DTO = mybir.dt.bfloat16       # out
NP1 = ml_dtypes.float8_e3m4
NP2 = ml_dtypes.float8_e3m4
OUTER_SPLIT = "ADADADAD"      # outer-product engine per j: A=scalar, D=vector

_built = {}


def _build(reps=1):
    """Build the Bass program. reps>1 repeats the whole workload in-kernel
    (used only for benchmarking; results identical)."""
    if reps in _built:
        return _built[reps]

    nc = bacc.Bacc("TRN2", target_bir_lowering=False, debug=False,
                   num_devices=N_CORES)
    x1 = nc.dram_tensor("x1", [BC * H, W], DT1, kind="ExternalInput")
    x2 = nc.dram_tensor("x2", [BC * H2, W2], DT2, kind="ExternalInput")
    # Tiny block-diagonal constants, fed from the host (engine memsets
    # can't write partition slices that start off 32-partition bounds).
    seld = nc.dram_tensor("sel", [128, PP], DT2, kind="ExternalInput")
    selSd = nc.dram_tensor("selS", [PP, 128], F32R, kind="ExternalInput")
    out = nc.dram_tensor("out", [BC * H, W], DTO, kind="ExternalOutput")

    # Packed views: block m covers pairs 4m..4m+3; partition p holds rows
    # [p*J1, (p+1)*J1) of the block's flattened row range -> per-partition
    # DRAM bytes are fully contiguous.
    x1v = x1.ap().rearrange("(m p j) w -> m p j w", p=128, j=J1)
    x2v = x2.ap().rearrange("(m p c) w -> m p c w", p=128, c=C2)
    outv = out.ap().rearrange("(m p j) w -> m p j w", p=128, j=J1)

    with TileContext(nc) as tc:
        with (
            tc.tile_pool(name="const", bufs=1) as cpool,
            tc.tile_pool(name="x1p", bufs=LEAD + 2) as x1pool,
            tc.tile_pool(name="rsp", bufs=LEAD + 2) as rspool,
            tc.tile_pool(name="x2p", bufs=4) as x2pool,
            tc.tile_pool(name="csb", bufs=4) as csbpool,
            tc.tile_pool(name="m2p", bufs=4) as m2pool,
            tc.tile_pool(name="op", bufs=4) as opool,
            tc.tile_pool(name="csp", bufs=3, space="PSUM") as cspool,
            tc.tile_pool(name="pbp", bufs=3, space="PSUM") as pbpool,
        ):
            sel = cpool.tile([128, PP], DT2)
            nc.sync.dma_start(out=sel[:], in_=seld.ap())
            selS = cpool.tile([PP, 128], F32R)
            nc.sync.dma_start(out=selS[:], in_=selSd.ap())

            for _rep in range(reps):
              rs_q = {}
              for mm in range(NBLK + LEAD):
                # Stage A (LEAD blocks ahead): x1 load + rowsums.
                if mm < NBLK:
                    x1t = x1pool.tile([128, J1, W], DT1)
                    nc.sync.dma_start(out=x1t[:], in_=x1v[mm])
                    rs = rspool.tile([128, J1], F32)
                    nc.vector.reduce_sum(out=rs[:], in_=x1t[:],
                                         axis=mybir.AxisListType.X)
                    rs_q[mm] = rs
                if mm < LEAD:
                    continue
                m = mm - LEAD
                rs = rs_q.pop(m)

                x2t = x2pool.tile([128, C2, W2], DT2)
                nc.sync.dma_start(out=x2t[:], in_=x2v[m])

                # Per-pair column sums over all 512 rows -> PSUM [PP, 512].
                cs = cspool.tile([PP, W2], F32)
                for c in range(C2):
                    nc.tensor.matmul(cs[:], lhsT=sel[:], rhs=x2t[:, c, :],
                                     start=(c == 0), stop=(c == C2 - 1))

                # PSUM -> SBUF on the scalar engine, then pair-add adjacent
                # cols (avgpool) on the vector engine.
                csb = csbpool.tile([PP, W2], F32)
                nc.scalar.activation(csb[:], cs[:],
                                     mybir.ActivationFunctionType.Copy)
                m2 = m2pool.tile([PP, W], F32R)
                csv = csb[:].rearrange("p (w t) -> p w t", t=2)
                nc.vector.tensor_add(m2[:], csv[:, :, 0], csv[:, :, 1])

                # Scale + broadcast each pair's m2 onto its 32 partitions.
                pb = pbpool.tile([128, W], F32)
                nc.tensor.matmul(pb[:], lhsT=selS[:], rhs=m2[:],
                                 start=True, stop=True)

                # Outer product: per-partition scalar multiply.
                ot = opool.tile([128, J1, W], DTO)
                for j in range(J1):
                    if OUTER_SPLIT[j % len(OUTER_SPLIT)] == "A":
                        nc.scalar.activation(
                            ot[:, j, :], pb[:],
                            mybir.ActivationFunctionType.Copy,
                            scale=rs[:, j:j + 1])
                    else:
                        nc.vector.tensor_scalar_mul(
                            ot[:, j, :], pb[:], rs[:, j:j + 1])
                nc.scalar.dma_start(out=outv[m], in_=ot[:])

    nc.compile()
    _built[reps] = nc
    return nc


def _sel_consts():
    sel = np.zeros((128, PP), dtype=NP2)
    selS = np.zeros((PP, 128), dtype=np.float32)
    for t in range(PP):
        sel[t * PB:(t + 1) * PB, t] = NP2(1.0)
        selS[t, t * PB:(t + 1) * PB] = SCALE
    return sel, selS


def _in_maps(x1, x2):
    x1 = np.asarray(x1, dtype=np.float32).astype(NP1)
    x2 = np.asarray(x2, dtype=np.float32).astype(NP2)
    sel, selS = _sel_consts()
    maps = []
    for i in range(N_CORES):
        maps.append({
            "x1": np.ascontiguousarray(
                x1[i * B_LOC:(i + 1) * B_LOC].reshape(BC * H, W)),
            "x2": np.ascontiguousarray(
                x2[i * B_LOC:(i + 1) * B_LOC].reshape(BC * H2, W2)),
            "sel": sel,
            "selS": selS,
        })
    return maps


def _run(x1, x2, **kw):
    nc = _build()
    return run_bass_kernel_spmd(nc, _in_maps(x1, x2), list(range(N_CORES)), **kw)


def kernel(x1, x2):
    res = _run(x1, x2)
    outs = [res.results[i]["out"].astype(np.float32).reshape(B_LOC, C, H, W)
            for i in range(N_CORES)]
    return np.concatenate(outs, axis=0)
